# revision 31
# baseline (speedup 1.0000x reference)
"""TRN2 Bass kernel for nn_ClassAttention (1x1 conv + BN + ReLU + windowed attention).

kernel(**inputs) takes FULL inputs, returns the FULL output [4,256,256,256] f32.
Shards data-parallel over (batch, image-row-half) across 8 NeuronCores, runs a
Bass/Tile SPMD program via run_bass_kernel_spmd, and unshards on the host.

Per-core shard (core = (b, rh) = (core//2, core%2)):
  x_sh   [256c, 16hh, 2048]   x[b,:,128rh:+128,:] rearranged window-contiguous:
                              [c, hh, (pw, win, r1, r2)]
  at_sh  [16hh, 128, 16384]   attn pre-transposed [pair, 64*win+k, 64*nh+q],
                              stored partition-major per row of windows
  w_prep [256c, 256o]         (w_conv * inv_std[:,None]).T  (BN scale folded)
  bias   [128, 256]           (beta - mean*inv_std) broadcast over partitions
  out    [16hh, 128p, 4096]   raw staging dump; host decodes
                              p = 32q+16win+d, f = u*1024+r1*128+pw*8+r2,
                              ch = 64u+16q+d

On-chip pipeline per window-pair (2 windows of 64 pixels, pixels on partitions):
  conv (PE): psum[128pix=(win,r1,r2), 256ch] = x_pair.T @ w_prep
             2 matmuls (K=128 halves), M=128, N=256, fp32
  bias (DVE): tmp = psum + bias_tile
  relu (ACT): block-diagonal V [128, (nh,win,d)]: diag cells = relu(tmp),
              off-diag cells stay zero (zeroed once at start, never rewritten)
  attn (PE): per head nh: one matmul computes BOTH windows via block-diag V:
             out[32,64] = V[:,32nh:+32].T @ At[:,64nh:+64], K=128, N=64,
             tile_position=(0, 32*(nh%4)) -> 4 column-tiles packed in the array
  evac (DVE): psum [128,(u,r1,r2)] -> staging [128, 4096]
  store (ACT hwdge ring): staging -> DRAM, 2 MiB contiguous per row of windows
"""

import numpy as np
from contextlib import ExitStack

import concourse.bacc as bacc
import concourse.tile as tile
import concourse.mybir as mybir
from concourse.bass_utils import run_bass_kernel_spmd

F32 = mybir.dt.float32
F16 = mybir.dt.float16
I8 = mybir.dt.int8
RELU = mybir.ActivationFunctionType.Relu

EPS = 1e-5
NCORES = 8

_cached_nc = None


def _build_program(n_vbd=4, at_bufs=4, G=8):
    nc = bacc.Bacc("TRN2", target_bir_lowering=False, debug=False)

    x_d = nc.dram_tensor("x_sh", [128, 16, 4096], F16, kind="ExternalInput")
    at_d = nc.dram_tensor("at_sh", [16, 128, 16384], I8, kind="ExternalInput")
    wc_d = nc.dram_tensor("w_prep", [256, 256], F16, kind="ExternalInput")
    b_d = nc.dram_tensor("bias", [128, 1024], F32, kind="ExternalInput")
    out_d = nc.dram_tensor("out_sh", [16, 128, 4096], F16, kind="ExternalOutput")

    ngroups = 16 // G        # at tiles per hh row
    GB = 4                   # pairs per elementwise batch group
    nbatch = G // GB         # batch groups per at tile

    with tile.TileContext(nc) as tc, ExitStack() as ctx:
        const = ctx.enter_context(tc.tile_pool(name="const", bufs=1))
        xp = ctx.enter_context(tc.tile_pool(name="xp", bufs=3))
        atp = ctx.enter_context(tc.tile_pool(name="atp", bufs=at_bufs))
        vbdp = ctx.enter_context(tc.tile_pool(name="vbdp", bufs=1))
        tvp = ctx.enter_context(tc.tile_pool(name="tvp", bufs=4))
        stp = ctx.enter_context(tc.tile_pool(name="stp", bufs=3))
        pscp = ctx.enter_context(tc.tile_pool(name="pscp", bufs=2, space="PSUM"))
        psap = ctx.enter_context(tc.tile_pool(name="psap", bufs=2, space="PSUM"))

        # const loads go on the scalar HWDGE ring so the sync ring's FIFO
        # starts with the bulk at/x loads immediately
        w0 = const.tile([128, 256], F16, name="w0")
        w1 = const.tile([128, 256], F16, name="w1")
        nc.scalar.dma_start(out=w0, in_=wc_d[0:128, :])
        nc.scalar.dma_start(out=w1, in_=wc_d[128:256, :])
        bias = const.tile([128, 1024], F32, name="bias_t")
        nc.scalar.dma_start(out=bias, in_=b_d[:, :])

        # Block-diagonal V tiles for GB pairs each: columns =
        # (pair GB, nh 16, win 2, d 16). Zeroed once; the relu writes only the
        # diagonal cells (win0 -> rows 0:64 of win-0 columns, win1 -> rows
        # 64:128 of win-1 columns), so the zeros persist across reuse and each
        # V[:, 512p+32nh:+32] is exactly block-diag(V0, V1).
        vbd = []
        for i in range(n_vbd):
            t = vbdp.tile([128, 512 * GB], F16, tag=f"vbd{i}", name=f"vbd{i}")
            nc.vector.memset(t, 0.0)
            vbd.append(t)
        vbd_i = 0

        for hh in range(16):
            xt = xp.tile([128, 4096], F16, tag="xt", name=f"xt_{hh}")
            nc.sync.dma_start(out=xt, in_=x_d[:, hh, :])

            st = stp.tile([128, 4096], F16, tag="st", name=f"st_{hh}")

            for g in range(ngroups):
                # SWDGE casting DMA: HBM int8 -> SBUF fp16 (ints <=127 are
                # exact in fp16; per-row dequant scales applied on host)
                at = atp.tile([128, 1024 * G], F16, tag="at", name=f"at_{hh}_{g}")
                nc.gpsimd.dma_start(
                    out=at,
                    in_=at_d[hh, :, 1024 * G * g: 1024 * G * (g + 1)])

                for bg in range(nbatch):
                    grp = nbatch * g + bg          # batch-group index in hh
                    ps4 = pscp.tile([128, 256 * GB], F32, tag="ps4",
                                    name=f"ps4_{hh}_{grp}")
                    for p in range(GB):
                        p16 = GB * grp + p         # pair index in hh
                        xsl0 = slice(128 * p16, 128 * p16 + 128)
                        xsl1 = slice(2048 + 128 * p16, 2048 + 128 * p16 + 128)
                        osl = slice(256 * p, 256 * p + 256)
                        nc.tensor.matmul(ps4[:, osl], xt[:, xsl0], w0,
                                         start=True, stop=False)
                        nc.tensor.matmul(ps4[:, osl], xt[:, xsl1], w1,
                                         start=False, stop=True)
                    tv4 = tvp.tile([128, 256 * GB], F16, tag="tv4",
                                   name=f"tv4_{hh}_{grp}")
                    nc.vector.tensor_add(tv4, ps4, bias)
                    V4 = vbd[vbd_i % n_vbd]
                    vbd_i += 1
                    Vr = V4.rearrange("pt (p nh two d) -> pt p nh two d",
                                      p=GB, nh=16, two=2, d=16)
                    tvr = tv4.rearrange("pt (p a b) -> pt p a b", p=GB, a=16)
                    nc.scalar.activation(Vr[0:64, :, :, 0, :], tvr[0:64], RELU)
                    nc.scalar.activation(Vr[64:128, :, :, 1, :], tvr[64:128],
                                         RELU)

                    pa4 = psap.tile([128, 256 * GB], F32, tag="pa4",
                                    name=f"pa4_{hh}_{grp}")
                    for p in range(GB):
                        ploc = GB * bg + p         # pair index in at tile
                        for j in range(4):
                            for quad in range(4):
                                nh = 4 * j + quad
                                nc.tensor.matmul(
                                    pa4[32 * quad:32 * quad + 32,
                                        256 * p + 64 * j:256 * p + 64 * j + 64],
                                    V4[:, 512 * p + 32 * nh:
                                       512 * p + 32 * nh + 32],
                                    at[:, 1024 * ploc + 64 * nh:
                                       1024 * ploc + 64 * nh + 64],
                                    start=True, stop=True,
                                    tile_position=(0, 32 * quad))
                    nc.vector.tensor_copy(
                        st[:, 1024 * grp:1024 * grp + 1024], pa4)
                    # half-row stores: 1 MiB each, shrinks the final
                    # un-overlapped store tail
                    if grp == 1:
                        nc.scalar.dma_start(out=out_d[hh, :, 0:2048],
                                            in_=st[:, 0:2048])
                    elif grp == 3:
                        nc.scalar.dma_start(out=out_d[hh, :, 2048:4096],
                                            in_=st[:, 2048:4096])

    nc.compile()
    return nc


def _shard_inputs(x, attn_i, w_conv, bn_gamma, bn_beta, bn_mean, bn_var):
    inv_std = (bn_gamma / np.sqrt(bn_var + np.float32(EPS))).astype(np.float32)
    shift = (bn_beta - bn_mean * inv_std).astype(np.float32)
    bias_tile = np.ascontiguousarray(
        np.broadcast_to(np.tile(shift, 4)[None, :], (128, 1024))
    ).astype(np.float32)
    w_prep = np.ascontiguousarray(
        (w_conv * inv_std[:, None]).T).astype(np.float16)
    x16 = x.astype(np.float16)
    # int8 per-row quantization of attn: a8[w,nh,q,:] = rint(attn/s), with
    # s = rowmax/127; the dequant multiply happens on the host in
    # _unshard_output (raw kernel output is in quantized units)
    amax = np.maximum(np.abs(attn_i).max(axis=3, keepdims=True), 1e-9)
    s_a = (amax / np.float32(127.0)).astype(np.float32)  # [4096, 16, 64, 1]
    a8 = np.rint(attn_i / s_a).astype(np.int8)
    in_maps = []
    scales = []
    for core in range(NCORES):
        b, rh = core // 2, core % 2
        x_sh = x16[b, :, 128 * rh:128 * rh + 128, :]
        # [c, hh, (pw, win, r1, r2)] then split c -> (half, cl) and move half
        # into the free axis: [cl, hh, (half, pw, win, r1, r2)]
        x_sh = np.ascontiguousarray(
            x_sh.reshape(2, 128, 16, 8, 16, 2, 8).transpose(1, 2, 0, 4, 5, 3, 6)
        ).reshape(128, 16, 4096)
        a_sl = a8[1024 * b + 512 * rh: 1024 * b + 512 * rh + 512]
        # [pair, 64win+k, 64nh+q], then partition-major per hh row
        # ([hh, p, pr, 1024]) so each at-load reads 4KiB/partition contiguous
        a_prep = a_sl.reshape(256, 2, 16, 64, 64).transpose(0, 1, 4, 2, 3) \
            .reshape(16, 16, 128, 1024)
        a_prep = np.ascontiguousarray(
            a_prep.transpose(0, 2, 1, 3)).reshape(16, 128, 16384)
        scales.append(s_a[1024 * b + 512 * rh: 1024 * b + 512 * rh + 512])
        in_maps.append(dict(x_sh=x_sh, at_sh=a_prep, w_prep=w_prep, bias=bias_tile))
    return in_maps, scales


def _unshard_output(results, scales):
    out = np.empty((4, 256, 256, 256), np.float32)
    for core in range(NCORES):
        b, rh = core // 2, core % 2
        raw = np.asarray(results[core]["out_sh"], np.float32)  # [16, 128, 4096]
        # partition = (quad4, win2, d16); f = pw*256 + j*64 + ws1*8 + ws2
        r = raw.reshape(16, 4, 2, 16, 16, 4, 8, 8)  # hh,quad,win,d,pw,j,ws1,ws2
        # dequant: window = (hh, pw), head nh = 4j+quad, q = 8*ws1+ws2
        # scales[core] is [512=(hh,pw), 2? no: [512 win, 16 nh, 64 q, 1]
        s = scales[core].reshape(16, 16, 2, 16, 8, 8)  # hh,pw,win,nh,ws1,ws2
        s = s.reshape(16, 16, 2, 4, 4, 8, 8)           # hh,pw,win,j,quad,ws1,ws2
        # -> [hh, quad, win, 1(d), pw, j, ws1, ws2]
        s = s.transpose(0, 4, 2, 1, 3, 5, 6)[:, :, :, None]
        r = r * s
        # ch = 16*(4j+quad)+d ; h = 8hh+ws1 ; w = 16pw+8win+ws2
        oc = r.transpose(5, 1, 3, 0, 6, 4, 2, 7).reshape(256, 128, 256)
        out[b, :, 128 * rh:128 * rh + 128, :] = oc
    return out


def get_program():
    global _cached_nc
    if _cached_nc is None:
        _cached_nc = _build_program()
    return _cached_nc


def run_sharded(in_maps, trace=False, **kwargs):
    nc = get_program()
    return run_bass_kernel_spmd(nc, in_maps, list(range(NCORES)),
                                trace=trace, **kwargs)


def kernel(x, attn_i, w_conv, bn_gamma, bn_beta, bn_mean, bn_var):
    x = np.asarray(x, dtype=np.float32)
    attn_i = np.asarray(attn_i, dtype=np.float32)
    w_conv = np.asarray(w_conv, dtype=np.float32)
    bn_gamma = np.asarray(bn_gamma, dtype=np.float32)
    bn_beta = np.asarray(bn_beta, dtype=np.float32)
    bn_mean = np.asarray(bn_mean, dtype=np.float32)
    bn_var = np.asarray(bn_var, dtype=np.float32)
    in_maps, scales = _shard_inputs(x, attn_i, w_conv, bn_gamma, bn_beta,
                                    bn_mean, bn_var)
    res = run_sharded(in_maps)
    return _unshard_output(res.results, scales)



# revision 35
# speedup vs baseline: 1.0294x; 1.0294x over previous
"""TRN2 Bass kernel for nn_ClassAttention (1x1 conv + BN + ReLU + windowed attention).

kernel(**inputs) takes FULL inputs, returns the FULL output [4,256,256,256] f32.
Shards data-parallel over (batch, image-row-half) across 8 NeuronCores, runs a
Bass/Tile SPMD program via run_bass_kernel_spmd, and unshards on the host.

Per-core shard (core = (b, rh) = (core//2, core%2)):
  x_sh   [256c, 16hh, 2048]   x[b,:,128rh:+128,:] rearranged window-contiguous:
                              [c, hh, (pw, win, r1, r2)]
  at_sh  [16hh, 128, 16384]   attn pre-transposed [pair, 64*win+k, 64*nh+q],
                              stored partition-major per row of windows
  w_prep [256c, 256o]         (w_conv * inv_std[:,None]).T  (BN scale folded)
  bias   [128, 256]           (beta - mean*inv_std) broadcast over partitions
  out    [16hh, 128p, 4096]   raw staging dump; host decodes
                              p = 32q+16win+d, f = u*1024+r1*128+pw*8+r2,
                              ch = 64u+16q+d

On-chip pipeline per window-pair (2 windows of 64 pixels, pixels on partitions):
  conv (PE): psum[128pix=(win,r1,r2), 256ch] = x_pair.T @ w_prep
             2 matmuls (K=128 halves), M=128, N=256, fp32
  bias (DVE): tmp = psum + bias_tile
  relu (ACT): block-diagonal V [128, (nh,win,d)]: diag cells = relu(tmp),
              off-diag cells stay zero (zeroed once at start, never rewritten)
  attn (PE): per head nh: one matmul computes BOTH windows via block-diag V:
             out[32,64] = V[:,32nh:+32].T @ At[:,64nh:+64], K=128, N=64,
             tile_position=(0, 32*(nh%4)) -> 4 column-tiles packed in the array
  evac (DVE): psum [128,(u,r1,r2)] -> staging [128, 4096]
  store (ACT hwdge ring): staging -> DRAM, 2 MiB contiguous per row of windows
"""

import numpy as np
from contextlib import ExitStack

import concourse.bacc as bacc
import concourse.tile as tile
import concourse.mybir as mybir
from concourse.bass_utils import run_bass_kernel_spmd

F32 = mybir.dt.float32
F16 = mybir.dt.float16
I8 = mybir.dt.int8
RELU = mybir.ActivationFunctionType.Relu

EPS = 1e-5
NCORES = 8

_cached_nc = None


def _build_program(n_vbd=4, at_bufs=2, G=16):
    nc = bacc.Bacc("TRN2", target_bir_lowering=False, debug=False)

    x_d = nc.dram_tensor("x_sh", [128, 16, 4096], F16, kind="ExternalInput")
    at8_d = nc.dram_tensor("at8_sh", [16, 128, 8192], I8, kind="ExternalInput")
    at16_d = nc.dram_tensor("at16_sh", [16, 128, 8192], F16, kind="ExternalInput")
    wc_d = nc.dram_tensor("w_prep", [256, 256], F16, kind="ExternalInput")
    b_d = nc.dram_tensor("bias", [128, 1024], F32, kind="ExternalInput")
    out_d = nc.dram_tensor("out_sh", [16, 128, 4096], F16, kind="ExternalOutput")

    ngroups = 16 // G        # at tiles per hh row
    GB = 4                   # pairs per elementwise batch group
    nbatch = G // GB         # batch groups per at tile

    with tile.TileContext(nc) as tc, ExitStack() as ctx:
        const = ctx.enter_context(tc.tile_pool(name="const", bufs=1))
        xp = ctx.enter_context(tc.tile_pool(name="xp", bufs=3))
        at8p = ctx.enter_context(tc.tile_pool(name="at8p", bufs=at_bufs))
        at16p = ctx.enter_context(tc.tile_pool(name="at16p", bufs=at_bufs))
        at8wp = ctx.enter_context(tc.tile_pool(name="at8wp", bufs=at_bufs))
        vbdp = ctx.enter_context(tc.tile_pool(name="vbdp", bufs=1))
        tvp = ctx.enter_context(tc.tile_pool(name="tvp", bufs=4))
        stp = ctx.enter_context(tc.tile_pool(name="stp", bufs=3))
        pscp = ctx.enter_context(tc.tile_pool(name="pscp", bufs=2, space="PSUM"))
        psap = ctx.enter_context(tc.tile_pool(name="psap", bufs=2, space="PSUM"))

        # const loads go on the scalar HWDGE ring so the sync ring's FIFO
        # starts with the bulk at/x loads immediately
        w0 = const.tile([128, 256], F16, name="w0")
        w1 = const.tile([128, 256], F16, name="w1")
        nc.scalar.dma_start(out=w0, in_=wc_d[0:128, :])
        nc.scalar.dma_start(out=w1, in_=wc_d[128:256, :])
        bias = const.tile([128, 1024], F32, name="bias_t")
        nc.scalar.dma_start(out=bias, in_=b_d[:, :])

        # Block-diagonal V tiles for GB pairs each: columns =
        # (pair GB, nh 16, win 2, d 16). Zeroed once; the relu writes only the
        # diagonal cells (win0 -> rows 0:64 of win-0 columns, win1 -> rows
        # 64:128 of win-1 columns), so the zeros persist across reuse and each
        # V[:, 512p+32nh:+32] is exactly block-diag(V0, V1).
        vbd = []
        for i in range(n_vbd):
            t = vbdp.tile([128, 512 * GB], F16, tag=f"vbd{i}", name=f"vbd{i}")
            nc.vector.memset(t, 0.0)
            vbd.append(t)
        vbd_i = 0

        for hh in range(16):
            xt = xp.tile([128, 4096], F16, tag="xt", name=f"xt_{hh}")
            nc.sync.dma_start(out=xt, in_=x_d[:, hh, :])

            st = stp.tile([128, 4096], F16, tag="st", name=f"st_{hh}")

            for g in range(ngroups):
                # heads 0-7 arrive int8 (per-row scaled on host; dequant on
                # host during decode); DVE widens int8 -> fp16 (ints exact)
                at8 = at8p.tile([128, 8192], I8, tag="at8", name=f"at8_{hh}")
                nc.sync.dma_start(out=at8, in_=at8_d[hh])
                at8w = at8wp.tile([128, 8192], F16, tag="at8w",
                                  name=f"at8w_{hh}")
                nc.vector.tensor_copy(at8w, at8)
                # heads 8-15 arrive as plain fp16
                at16 = at16p.tile([128, 8192], F16, tag="at16",
                                  name=f"at16_{hh}")
                nc.sync.dma_start(out=at16, in_=at16_d[hh])

                for bg in range(nbatch):
                    grp = nbatch * g + bg          # batch-group index in hh
                    ps4 = pscp.tile([128, 256 * GB], F32, tag="ps4",
                                    name=f"ps4_{hh}_{grp}")
                    for p in range(GB):
                        p16 = GB * grp + p         # pair index in hh
                        xsl0 = slice(128 * p16, 128 * p16 + 128)
                        xsl1 = slice(2048 + 128 * p16, 2048 + 128 * p16 + 128)
                        osl = slice(256 * p, 256 * p + 256)
                        nc.tensor.matmul(ps4[:, osl], xt[:, xsl0], w0,
                                         start=True, stop=False)
                        nc.tensor.matmul(ps4[:, osl], xt[:, xsl1], w1,
                                         start=False, stop=True)
                    tv4 = tvp.tile([128, 256 * GB], F16, tag="tv4",
                                   name=f"tv4_{hh}_{grp}")
                    nc.vector.tensor_add(tv4, ps4, bias)
                    V4 = vbd[vbd_i % n_vbd]
                    vbd_i += 1
                    Vr = V4.rearrange("pt (p nh two d) -> pt p nh two d",
                                      p=GB, nh=16, two=2, d=16)
                    tvr = tv4.rearrange("pt (p a b) -> pt p a b", p=GB, a=16)
                    nc.scalar.activation(Vr[0:64, :, :, 0, :], tvr[0:64], RELU)
                    nc.scalar.activation(Vr[64:128, :, :, 1, :], tvr[64:128],
                                         RELU)

                    pa4 = psap.tile([128, 256 * GB], F32, tag="pa4",
                                    name=f"pa4_{hh}_{grp}")
                    for p in range(GB):
                        ploc = GB * bg + p         # pair index in at tile
                        for j in range(4):
                            for quad in range(4):
                                nh = 4 * j + quad
                                nc.tensor.matmul(
                                    pa4[32 * quad:32 * quad + 32,
                                        256 * p + 64 * j:256 * p + 64 * j + 64],
                                    V4[:, 512 * p + 32 * nh:
                                       512 * p + 32 * nh + 32],
                                    (at8w[:, 512 * ploc + 64 * nh:
                                          512 * ploc + 64 * nh + 64]
                                     if nh < 8 else
                                     at16[:, 512 * ploc + 64 * (nh - 8):
                                          512 * ploc + 64 * (nh - 8) + 64]),
                                    start=True, stop=True,
                                    tile_position=(0, 32 * quad))
                    nc.vector.tensor_copy(
                        st[:, 1024 * grp:1024 * grp + 1024], pa4)
                    # half-row stores: 1 MiB each, shrinks the final
                    # un-overlapped store tail
                    if grp == 1:
                        nc.scalar.dma_start(out=out_d[hh, :, 0:2048],
                                            in_=st[:, 0:2048])
                    elif grp == 3:
                        nc.scalar.dma_start(out=out_d[hh, :, 2048:4096],
                                            in_=st[:, 2048:4096])

    nc.compile()
    return nc


def _shard_inputs(x, attn_i, w_conv, bn_gamma, bn_beta, bn_mean, bn_var):
    inv_std = (bn_gamma / np.sqrt(bn_var + np.float32(EPS))).astype(np.float32)
    shift = (bn_beta - bn_mean * inv_std).astype(np.float32)
    bias_tile = np.ascontiguousarray(
        np.broadcast_to(np.tile(shift, 4)[None, :], (128, 1024))
    ).astype(np.float32)
    w_prep = np.ascontiguousarray(
        (w_conv * inv_std[:, None]).T).astype(np.float16)
    x16 = x.astype(np.float16)
    # heads 0-7: int8 with per-row scales (dequant on host in decode);
    # heads 8-15: plain fp16
    a_lo = attn_i[:, 0:8]
    amax = np.maximum(np.abs(a_lo).max(axis=3, keepdims=True), 1e-9)
    s_a = (amax / np.float32(127.0)).astype(np.float32)  # [4096, 8, 64, 1]
    a8 = np.rint(a_lo / s_a).astype(np.int8)
    a16 = attn_i[:, 8:16].astype(np.float16)
    in_maps = []
    scales = []
    for core in range(NCORES):
        b, rh = core // 2, core % 2
        x_sh = x16[b, :, 128 * rh:128 * rh + 128, :]
        # [c, hh, (pw, win, r1, r2)] then split c -> (half, cl) and move half
        # into the free axis: [cl, hh, (half, pw, win, r1, r2)]
        x_sh = np.ascontiguousarray(
            x_sh.reshape(2, 128, 16, 8, 16, 2, 8).transpose(1, 2, 0, 4, 5, 3, 6)
        ).reshape(128, 16, 4096)
        a_sl = at16[1024 * b + 512 * rh: 1024 * b + 512 * rh + 512]
        # [pair, 64win+k, 64nh+q], then partition-major per hh row
        # ([hh, p, pr, 1024]) so each at-load reads 4KiB/partition contiguous
        a_prep = a_sl.reshape(256, 2, 16, 64, 64).transpose(0, 1, 4, 2, 3) \
            .reshape(16, 16, 128, 1024)
        a_prep = np.ascontiguousarray(
            a_prep.transpose(0, 2, 1, 3)).reshape(16, 128, 16384)
        in_maps.append(dict(x_sh=x_sh, at_sh=a_prep, w_prep=w_prep, bias=bias_tile))
    return in_maps


def _unshard_output(results, scales):
    out = np.empty((4, 256, 256, 256), np.float32)
    for core in range(NCORES):
        b, rh = core // 2, core % 2
        raw = np.asarray(results[core]["out_sh"], np.float32)  # [16, 128, 4096]
        # partition = (quad4, win2, d16); f = pw*256 + j*64 + ws1*8 + ws2
        r = raw.reshape(16, 4, 2, 16, 16, 4, 8, 8)  # hh,quad,win,d,pw,j,ws1,ws2
        # dequant heads 0-7 (j<2): scales[core] [512=(hh,pw,win), 8nh, 64q, 1]
        s = scales[core].reshape(16, 16, 2, 2, 4, 8, 8)  # hh,pw,win,j,quad,ws1,ws2
        # -> [hh, quad, win, 1(d), pw, j, ws1, ws2]
        s = s.transpose(0, 4, 2, 1, 3, 5, 6)[:, :, :, None]
        r = r.copy()
        r[:, :, :, :, :, 0:2] *= s
        # ch = 16*(4j+quad)+d ; h = 8hh+ws1 ; w = 16pw+8win+ws2
        oc = r.transpose(5, 1, 3, 0, 6, 4, 2, 7).reshape(256, 128, 256)
        out[b, :, 128 * rh:128 * rh + 128, :] = oc
    return out


def get_program():
    global _cached_nc
    if _cached_nc is None:
        _cached_nc = _build_program()
    return _cached_nc


def run_sharded(in_maps, trace=False, **kwargs):
    nc = get_program()
    return run_bass_kernel_spmd(nc, in_maps, list(range(NCORES)),
                                trace=trace, **kwargs)


def kernel(x, attn_i, w_conv, bn_gamma, bn_beta, bn_mean, bn_var):
    x = np.asarray(x, dtype=np.float32)
    attn_i = np.asarray(attn_i, dtype=np.float32)
    w_conv = np.asarray(w_conv, dtype=np.float32)
    bn_gamma = np.asarray(bn_gamma, dtype=np.float32)
    bn_beta = np.asarray(bn_beta, dtype=np.float32)
    bn_mean = np.asarray(bn_mean, dtype=np.float32)
    bn_var = np.asarray(bn_var, dtype=np.float32)
    in_maps, scales = _shard_inputs(x, attn_i, w_conv, bn_gamma, bn_beta,
                                    bn_mean, bn_var)
    res = run_sharded(in_maps)
    return _unshard_output(res.results, scales)



# revision 36
# speedup vs baseline: 1.1849x; 1.1511x over previous
"""TRN2 Bass kernel for nn_ClassAttention (1x1 conv + BN + ReLU + windowed attention).

kernel(**inputs) takes FULL inputs, returns the FULL output [4,256,256,256] f32.
Shards data-parallel over (batch, image-row-half) across 8 NeuronCores, runs a
Bass/Tile SPMD program via run_bass_kernel_spmd, and unshards on the host.

Per-core shard (core = (b, rh) = (core//2, core%2)):
  x_sh   [256c, 16hh, 2048]   x[b,:,128rh:+128,:] rearranged window-contiguous:
                              [c, hh, (pw, win, r1, r2)]
  at_sh  [16hh, 128, 16384]   attn pre-transposed [pair, 64*win+k, 64*nh+q],
                              stored partition-major per row of windows
  w_prep [256c, 256o]         (w_conv * inv_std[:,None]).T  (BN scale folded)
  bias   [128, 256]           (beta - mean*inv_std) broadcast over partitions
  out    [16hh, 128p, 4096]   raw staging dump; host decodes
                              p = 32q+16win+d, f = u*1024+r1*128+pw*8+r2,
                              ch = 64u+16q+d

On-chip pipeline per window-pair (2 windows of 64 pixels, pixels on partitions):
  conv (PE): psum[128pix=(win,r1,r2), 256ch] = x_pair.T @ w_prep
             2 matmuls (K=128 halves), M=128, N=256, fp32
  bias (DVE): tmp = psum + bias_tile
  relu (ACT): block-diagonal V [128, (nh,win,d)]: diag cells = relu(tmp),
              off-diag cells stay zero (zeroed once at start, never rewritten)
  attn (PE): per head nh: one matmul computes BOTH windows via block-diag V:
             out[32,64] = V[:,32nh:+32].T @ At[:,64nh:+64], K=128, N=64,
             tile_position=(0, 32*(nh%4)) -> 4 column-tiles packed in the array
  evac (DVE): psum [128,(u,r1,r2)] -> staging [128, 4096]
  store (ACT hwdge ring): staging -> DRAM, 2 MiB contiguous per row of windows
"""

import numpy as np
from contextlib import ExitStack

import concourse.bacc as bacc
import concourse.tile as tile
import concourse.mybir as mybir
from concourse.bass_utils import run_bass_kernel_spmd

F32 = mybir.dt.float32
F16 = mybir.dt.float16
I8 = mybir.dt.int8
RELU = mybir.ActivationFunctionType.Relu

EPS = 1e-5
NCORES = 8

_cached_nc = None


def _build_program(n_vbd=4, at_bufs=2, G=16):
    nc = bacc.Bacc("TRN2", target_bir_lowering=False, debug=False)

    x_d = nc.dram_tensor("x_sh", [128, 16, 4096], F16, kind="ExternalInput")
    at8_d = nc.dram_tensor("at8_sh", [16, 128, 8192], I8, kind="ExternalInput")
    at16_d = nc.dram_tensor("at16_sh", [16, 128, 8192], F16, kind="ExternalInput")
    wc_d = nc.dram_tensor("w_prep", [256, 256], F16, kind="ExternalInput")
    b_d = nc.dram_tensor("bias", [128, 1024], F32, kind="ExternalInput")
    out_d = nc.dram_tensor("out_sh", [16, 128, 4096], F16, kind="ExternalOutput")

    ngroups = 16 // G        # at tiles per hh row
    GB = 4                   # pairs per elementwise batch group
    nbatch = G // GB         # batch groups per at tile

    with tile.TileContext(nc) as tc, ExitStack() as ctx:
        const = ctx.enter_context(tc.tile_pool(name="const", bufs=1))
        xp = ctx.enter_context(tc.tile_pool(name="xp", bufs=3))
        at8p = ctx.enter_context(tc.tile_pool(name="at8p", bufs=at_bufs))
        at16p = ctx.enter_context(tc.tile_pool(name="at16p", bufs=at_bufs))
        at8wp = ctx.enter_context(tc.tile_pool(name="at8wp", bufs=at_bufs))
        vbdp = ctx.enter_context(tc.tile_pool(name="vbdp", bufs=1))
        tvp = ctx.enter_context(tc.tile_pool(name="tvp", bufs=4))
        stp = ctx.enter_context(tc.tile_pool(name="stp", bufs=3))
        pscp = ctx.enter_context(tc.tile_pool(name="pscp", bufs=2, space="PSUM"))
        psap = ctx.enter_context(tc.tile_pool(name="psap", bufs=2, space="PSUM"))

        # const loads go on the scalar HWDGE ring so the sync ring's FIFO
        # starts with the bulk at/x loads immediately
        w0 = const.tile([128, 256], F16, name="w0")
        w1 = const.tile([128, 256], F16, name="w1")
        nc.scalar.dma_start(out=w0, in_=wc_d[0:128, :])
        nc.scalar.dma_start(out=w1, in_=wc_d[128:256, :])
        bias = const.tile([128, 1024], F32, name="bias_t")
        nc.scalar.dma_start(out=bias, in_=b_d[:, :])

        # Block-diagonal V tiles for GB pairs each: columns =
        # (pair GB, nh 16, win 2, d 16). Zeroed once; the relu writes only the
        # diagonal cells (win0 -> rows 0:64 of win-0 columns, win1 -> rows
        # 64:128 of win-1 columns), so the zeros persist across reuse and each
        # V[:, 512p+32nh:+32] is exactly block-diag(V0, V1).
        vbd = []
        for i in range(n_vbd):
            t = vbdp.tile([128, 512 * GB], F16, tag=f"vbd{i}", name=f"vbd{i}")
            nc.vector.memset(t, 0.0)
            vbd.append(t)
        vbd_i = 0

        for hh in range(16):
            xt = xp.tile([128, 4096], F16, tag="xt", name=f"xt_{hh}")
            nc.sync.dma_start(out=xt, in_=x_d[:, hh, :])

            st = stp.tile([128, 4096], F16, tag="st", name=f"st_{hh}")

            for g in range(ngroups):
                # heads 0-7 arrive int8 (per-row scaled on host; dequant on
                # host during decode); DVE widens int8 -> fp16 (ints exact)
                at8 = at8p.tile([128, 8192], I8, tag="at8", name=f"at8_{hh}")
                nc.sync.dma_start(out=at8, in_=at8_d[hh])
                at8w = at8wp.tile([128, 8192], F16, tag="at8w",
                                  name=f"at8w_{hh}")
                nc.vector.tensor_copy(at8w, at8)
                # heads 8-15 arrive as plain fp16
                at16 = at16p.tile([128, 8192], F16, tag="at16",
                                  name=f"at16_{hh}")
                nc.sync.dma_start(out=at16, in_=at16_d[hh])

                for bg in range(nbatch):
                    grp = nbatch * g + bg          # batch-group index in hh
                    ps4 = pscp.tile([128, 256 * GB], F32, tag="ps4",
                                    name=f"ps4_{hh}_{grp}")
                    for p in range(GB):
                        p16 = GB * grp + p         # pair index in hh
                        xsl0 = slice(128 * p16, 128 * p16 + 128)
                        xsl1 = slice(2048 + 128 * p16, 2048 + 128 * p16 + 128)
                        osl = slice(256 * p, 256 * p + 256)
                        nc.tensor.matmul(ps4[:, osl], xt[:, xsl0], w0,
                                         start=True, stop=False)
                        nc.tensor.matmul(ps4[:, osl], xt[:, xsl1], w1,
                                         start=False, stop=True)
                    tv4 = tvp.tile([128, 256 * GB], F16, tag="tv4",
                                   name=f"tv4_{hh}_{grp}")
                    nc.vector.tensor_add(tv4, ps4, bias)
                    V4 = vbd[vbd_i % n_vbd]
                    vbd_i += 1
                    Vr = V4.rearrange("pt (p nh two d) -> pt p nh two d",
                                      p=GB, nh=16, two=2, d=16)
                    tvr = tv4.rearrange("pt (p a b) -> pt p a b", p=GB, a=16)
                    nc.scalar.activation(Vr[0:64, :, :, 0, :], tvr[0:64], RELU)
                    nc.scalar.activation(Vr[64:128, :, :, 1, :], tvr[64:128],
                                         RELU)

                    pa4 = psap.tile([128, 256 * GB], F32, tag="pa4",
                                    name=f"pa4_{hh}_{grp}")
                    for p in range(GB):
                        ploc = GB * bg + p         # pair index in at tile
                        for j in range(4):
                            for quad in range(4):
                                nh = 4 * j + quad
                                nc.tensor.matmul(
                                    pa4[32 * quad:32 * quad + 32,
                                        256 * p + 64 * j:256 * p + 64 * j + 64],
                                    V4[:, 512 * p + 32 * nh:
                                       512 * p + 32 * nh + 32],
                                    (at8w[:, 512 * ploc + 64 * nh:
                                          512 * ploc + 64 * nh + 64]
                                     if nh < 8 else
                                     at16[:, 512 * ploc + 64 * (nh - 8):
                                          512 * ploc + 64 * (nh - 8) + 64]),
                                    start=True, stop=True,
                                    tile_position=(0, 32 * quad))
                    nc.vector.tensor_copy(
                        st[:, 1024 * grp:1024 * grp + 1024], pa4)
                    # half-row stores: 1 MiB each, shrinks the final
                    # un-overlapped store tail
                    if grp == 1:
                        nc.scalar.dma_start(out=out_d[hh, :, 0:2048],
                                            in_=st[:, 0:2048])
                    elif grp == 3:
                        nc.scalar.dma_start(out=out_d[hh, :, 2048:4096],
                                            in_=st[:, 2048:4096])

    nc.compile()
    return nc


def _shard_inputs(x, attn_i, w_conv, bn_gamma, bn_beta, bn_mean, bn_var):
    inv_std = (bn_gamma / np.sqrt(bn_var + np.float32(EPS))).astype(np.float32)
    shift = (bn_beta - bn_mean * inv_std).astype(np.float32)
    bias_tile = np.ascontiguousarray(
        np.broadcast_to(np.tile(shift, 4)[None, :], (128, 1024))
    ).astype(np.float32)
    w_prep = np.ascontiguousarray(
        (w_conv * inv_std[:, None]).T).astype(np.float16)
    x16 = x.astype(np.float16)
    # heads 0-7: int8 with per-row scales (dequant on host in decode);
    # heads 8-15: plain fp16
    a_lo = attn_i[:, 0:8]
    amax = np.maximum(np.abs(a_lo).max(axis=3, keepdims=True), 1e-9)
    s_a = (amax / np.float32(127.0)).astype(np.float32)  # [4096, 8, 64, 1]
    a8 = np.rint(a_lo / s_a).astype(np.int8)
    a16 = attn_i[:, 8:16].astype(np.float16)
    in_maps = []
    scales = []
    for core in range(NCORES):
        b, rh = core // 2, core % 2
        x_sh = x16[b, :, 128 * rh:128 * rh + 128, :]
        # [c, hh, (pw, win, r1, r2)] then split c -> (half, cl) and move half
        # into the free axis: [cl, hh, (half, pw, win, r1, r2)]
        x_sh = np.ascontiguousarray(
            x_sh.reshape(2, 128, 16, 8, 16, 2, 8).transpose(1, 2, 0, 4, 5, 3, 6)
        ).reshape(128, 16, 4096)
        sl = slice(1024 * b + 512 * rh, 1024 * b + 512 * rh + 512)

        def prep(a):  # [512, 8, 64, 64] -> [16, 128, 8192], pair-transposed
            p = a.reshape(256, 2, 8, 64, 64).transpose(0, 1, 4, 2, 3) \
                .reshape(16, 16, 128, 512)
            return np.ascontiguousarray(
                p.transpose(0, 2, 1, 3)).reshape(16, 128, 8192)

        scales.append(s_a[sl])
        in_maps.append(dict(x_sh=x_sh, at8_sh=prep(a8[sl]),
                            at16_sh=prep(a16[sl]),
                            w_prep=w_prep, bias=bias_tile))
    return in_maps, scales


def _unshard_output(results, scales):
    out = np.empty((4, 256, 256, 256), np.float32)
    for core in range(NCORES):
        b, rh = core // 2, core % 2
        raw = np.asarray(results[core]["out_sh"], np.float32)  # [16, 128, 4096]
        # partition = (quad4, win2, d16); f = pw*256 + j*64 + ws1*8 + ws2
        r = raw.reshape(16, 4, 2, 16, 16, 4, 8, 8)  # hh,quad,win,d,pw,j,ws1,ws2
        # dequant heads 0-7 (j<2): scales[core] [512=(hh,pw,win), 8nh, 64q, 1]
        s = scales[core].reshape(16, 16, 2, 2, 4, 8, 8)  # hh,pw,win,j,quad,ws1,ws2
        # -> [hh, quad, win, 1(d), pw, j, ws1, ws2]
        s = s.transpose(0, 4, 2, 1, 3, 5, 6)[:, :, :, None]
        r = r.copy()
        r[:, :, :, :, :, 0:2] *= s
        # ch = 16*(4j+quad)+d ; h = 8hh+ws1 ; w = 16pw+8win+ws2
        oc = r.transpose(5, 1, 3, 0, 6, 4, 2, 7).reshape(256, 128, 256)
        out[b, :, 128 * rh:128 * rh + 128, :] = oc
    return out


def get_program():
    global _cached_nc
    if _cached_nc is None:
        _cached_nc = _build_program()
    return _cached_nc


def run_sharded(in_maps, trace=False, **kwargs):
    nc = get_program()
    return run_bass_kernel_spmd(nc, in_maps, list(range(NCORES)),
                                trace=trace, **kwargs)


def kernel(x, attn_i, w_conv, bn_gamma, bn_beta, bn_mean, bn_var):
    x = np.asarray(x, dtype=np.float32)
    attn_i = np.asarray(attn_i, dtype=np.float32)
    w_conv = np.asarray(w_conv, dtype=np.float32)
    bn_gamma = np.asarray(bn_gamma, dtype=np.float32)
    bn_beta = np.asarray(bn_beta, dtype=np.float32)
    bn_mean = np.asarray(bn_mean, dtype=np.float32)
    bn_var = np.asarray(bn_var, dtype=np.float32)
    in_maps, scales = _shard_inputs(x, attn_i, w_conv, bn_gamma, bn_beta,
                                    bn_mean, bn_var)
    res = run_sharded(in_maps)
    return _unshard_output(res.results, scales)

